# revision 1
# baseline (speedup 1.0000x reference)
"""GCN (3-layer GCNConv + linear head + log_softmax) on 8 TRN2 NeuronCores.

Sharding: nodes across 8 cores (node slots padded to NB*128 per core).
Per layer: local matmul h = x @ W (fp16), AllGather h (fp16),
then per dst-block edge aggregation: indirect-DMA gather of h[src] rows,
one-hot (norm-folded) matmul segment-sum into PSUM, ACT epilogue
(bias + optional relu) writing feature-major activations for the next layer.
"""
import math
import numpy as np

import concourse.bass as bass
import concourse.bacc as bacc
import concourse.tile as tile
from concourse import mybir

P = 128
f16 = mybir.dt.float16
f32 = mybir.dt.float32
i32 = mybir.dt.int32
AF = mybir.ActivationFunctionType
ALU = mybir.AluOpType

N_CORES = 8


# ----------------------------------------------------------------------------
# Host-side preprocessing
# ----------------------------------------------------------------------------

def sigmoid64(x):
    return 1.0 / (1.0 + np.exp(-x.astype(np.float64)))


def pack_nodes(n_nodes, weights, n_bins, cap, rng):
    """LPT-pack nodes into n_bins bins (<=cap nodes each), balancing sum of
    weights. Returns bin assignment [n_nodes]."""
    order = np.argsort(-weights, kind="stable")
    loads = np.zeros(n_bins)
    counts = np.zeros(n_bins, dtype=np.int64)
    assign = np.full(n_nodes, -1, dtype=np.int64)
    # heap-free greedy: argmin over bins each step would be O(n*b); use
    # vectorized approach via repeatedly taking argmin (fine at this scale)
    import heapq
    heap = [(0.0, b) for b in range(n_bins)]
    heapq.heapify(heap)
    full = set()
    for i in order:
        while True:
            load, b = heapq.heappop(heap)
            if b in full:
                continue
            break
        assign[i] = b
        loads[b] = load + weights[i]
        counts[b] += 1
        if counts[b] >= cap:
            full.add(b)
        else:
            heapq.heappush(heap, (loads[b], b))
    return assign, loads


def preprocess(x, edge_index, edge_w_params, n_blk=None):
    """Returns per-core input arrays + metadata for unpermuting outputs."""
    n = x.shape[0]
    e_src = edge_index[0].astype(np.int64)
    e_dst = edge_index[1].astype(np.int64)
    ne = e_src.shape[0]

    ew = sigmoid64(edge_w_params)
    deg = np.zeros(n, dtype=np.float64)
    np.add.at(deg, e_dst, ew)
    deg += 1.0
    dinv = 1.0 / np.sqrt(deg)
    indeg = np.bincount(e_dst, minlength=n).astype(np.float64) + 1.0
    dinv3 = 1.0 / np.sqrt(indeg)

    norm12 = (dinv[e_src] * ew * dinv[e_dst]).astype(np.float32)
    norm3 = (dinv3[e_src] * dinv3[e_dst]).astype(np.float32)
    self12 = (dinv * dinv).astype(np.float32)
    self3 = (dinv3 * dinv3).astype(np.float32)

    w = np.bincount(e_dst, minlength=n).astype(np.float64)  # in-degree (gather rows)
    rng = np.random.default_rng(0)

    if n_blk is None:
        n_blk = math.ceil(n / N_CORES / P)
    slots_per_core = n_blk * P
    half_slots = N_CORES * slots_per_core // 2

    # two-level packing: nodes -> 8 shards (cap slots_per_core), then within
    # each shard -> n_blk blocks (cap 128 nodes)
    shard, _ = pack_nodes(n, w, N_CORES, slots_per_core, rng)
    block = np.full(n, -1, dtype=np.int64)
    pos = np.full(n, -1, dtype=np.int64)
    t_g_need = 0
    for c in range(N_CORES):
        nodes_c = np.where(shard == c)[0]
        blk_c, loads_c = pack_nodes(len(nodes_c), w[nodes_c], n_blk, P, rng)
        block[nodes_c] = blk_c
        # positions within block
        for b in range(n_blk):
            nb = nodes_c[blk_c == b]
            block_rows = int(w[nb].sum())
            t_g_need = max(t_g_need, math.ceil(max(block_rows, 1) / P))
            pos[nb] = np.arange(len(nb))
    T_h = 5   # gather tiles per bank half
    T_g = 2 * T_h
    T = T_g + 1  # + self tile

    # global slot id
    slot = shard * slots_per_core + block * P + pos  # [n]

    # per-core arrays
    NF = x.shape[1]
    KF = NF // P
    xT = np.zeros((N_CORES, P, KF, slots_per_core), dtype=np.float16)
    dst_loc = np.zeros((N_CORES, P, n_blk * T), dtype=np.float16)
    nrm12_a = np.zeros((N_CORES, P, n_blk * T), dtype=np.float16)
    nrm3_a = np.zeros((N_CORES, P, n_blk * T), dtype=np.float16)

    xt_full = x.T.astype(np.float16)  # [NF, n]
    for c in range(N_CORES):
        nodes_c = np.where(shard == c)[0]
        sl = block[nodes_c] * P + pos[nodes_c]
        xc = np.zeros((NF, slots_per_core), dtype=np.float16)
        xc[:, sl] = xt_full[:, nodes_c]
        xT[c] = xc.reshape(KF, P, slots_per_core).transpose(1, 0, 2)

    # edges grouped by (shard(dst), block(dst)), split by src bank half
    NIH = T_h * P            # idxs per (block, half) dma_gather call
    CW = NIH // 16           # wrapped idx columns per call
    e_shard = shard[e_dst]
    e_block = block[e_dst]
    e_slotsrc = slot[e_src].astype(np.int32)
    e_dloc = pos[e_dst].astype(np.int64)
    src16 = np.zeros((N_CORES, P, n_blk * 2 * CW), dtype=np.int16)
    for c in range(N_CORES):
        for b in range(n_blk):
            mall = np.where((e_shard == c) & (e_block == b))[0]
            for half in range(2):
                if half == 0:
                    m = mall[e_slotsrc[mall] < half_slots]
                    loc = e_slotsrc[m]
                else:
                    m = mall[e_slotsrc[mall] >= half_slots]
                    loc = e_slotsrc[m] - half_slots
                k = len(m)
                assert k <= NIH, (c, b, half, k, NIH)
                order = np.argsort(loc, kind="stable")
                m, loc = m[order], loc[order]
                flat = np.zeros(NIH, dtype=np.int16)
                flat[:k] = loc.astype(np.int16)
                src16[c, :, (b * 2 + half) * CW:(b * 2 + half + 1) * CW] = \
                    np.tile(flat.reshape(CW, 16).T, (8, 1))
                sl = np.arange(k)
                tt = half * T_h + sl // P
                pp = sl % P
                colsT = b * T + tt
                dst_loc[c, pp, colsT] = e_dloc[m].astype(np.float16)
                nrm12_a[c, pp, colsT] = norm12[m].astype(np.float16)
                nrm3_a[c, pp, colsT] = norm3[m].astype(np.float16)
            # self tile (tile index T_g): position p <-> node at (c,b,p)
            nodes_cb = np.where((shard == c) & (block == b))[0]
            pcb = pos[nodes_cb]
            col_self = b * T + T_g
            dst_loc[c, pcb, col_self] = pcb.astype(np.float16)
            nrm12_a[c, pcb, col_self] = self12[nodes_cb].astype(np.float16)
            nrm3_a[c, pcb, col_self] = self3[nodes_cb].astype(np.float16)

    meta = dict(slot=slot, n_blk=n_blk, T_g=T_g, T_h=T_h,
                slots_per_core=slots_per_core)
    return dict(xT=xT, src16=src16, dst_loc=dst_loc, nrm12=nrm12_a,
                nrm3=nrm3_a), meta


def prep_weights(W1, b1, W2, b2, W3, b3, Wlin, blin):
    def wt(W):
        K, N = W.shape
        return W.astype(np.float16).reshape(K // P, P, N).transpose(1, 0, 2).copy()
    def bh(b):
        return b.astype(np.float32).reshape(-1, P).T.copy()  # [128, KH]
    return dict(w1=wt(W1), w2=wt(W2), w3=wt(W3), wlin=wt(Wlin),
                b1h=bh(b1), b2h=bh(b2), b3h=bh(b3),
                blin_rep=np.tile(blin.astype(np.float32)[None, :], (P, 1)))


# ----------------------------------------------------------------------------
# Kernel builder
# ----------------------------------------------------------------------------

def build_kernel(n_blk, T_g, NF=512, NH=256, NC=8):
    KF = NF // P
    KH = NH // P  # 2
    T = T_g + 1
    T_h = T_g // 2
    NIH = T_h * P
    CW = NIH // 16
    SPC = n_blk * P          # slots per core
    NTOT = N_CORES * SPC     # h_full rows
    HALF = NTOT // 2

    nc = bacc.Bacc("TRN2", target_bir_lowering=False, debug=False,
                   num_devices=N_CORES, num_swdge_queues=4)

    xT_in = nc.dram_tensor("xT", [P, KF * SPC], f16, kind="ExternalInput")
    s16_in = nc.dram_tensor("src16", [P, n_blk * 2 * CW], mybir.dt.int16, kind="ExternalInput")
    dst_in = nc.dram_tensor("dst_loc", [P, n_blk * T], f16, kind="ExternalInput")
    n12_in = nc.dram_tensor("nrm12", [P, n_blk * T], f16, kind="ExternalInput")
    n3_in = nc.dram_tensor("nrm3", [P, n_blk * T], f16, kind="ExternalInput")
    w1_in = nc.dram_tensor("w1", [P, KF * NH], f16, kind="ExternalInput")
    w2_in = nc.dram_tensor("w2", [P, KH * NH], f16, kind="ExternalInput")
    w3_in = nc.dram_tensor("w3", [P, KH * NH], f16, kind="ExternalInput")
    wl_in = nc.dram_tensor("wlin", [P, 3 * KH * NC], f16, kind="ExternalInput")
    b1_in = nc.dram_tensor("b1h", [P, KH], f32, kind="ExternalInput")
    b2_in = nc.dram_tensor("b2h", [P, KH], f32, kind="ExternalInput")
    b3_in = nc.dram_tensor("b3h", [P, KH], f32, kind="ExternalInput")
    bl_in = nc.dram_tensor("blin_rep", [P, NC], f32, kind="ExternalInput")
    out_t = nc.dram_tensor("out", [SPC, NC], f32, kind="ExternalOutput")

    hc = nc.dram_tensor("hc_bounce", [SPC, NH], f16, kind="Internal")
    hfull = nc.dram_tensor("hfull", [NTOT, NH], f16, kind="Internal",
                           addr_space="Shared")
    rg = [list(range(N_CORES))]

    with tile.TileContext(nc) as tc:
        with tc.tile_pool(name="state", bufs=1) as state, \
             tc.tile_pool(name="sbuf", bufs=4) as sbuf, \
             tc.tile_pool(name="spool", bufs=2) as spool, \
             tc.tile_pool(name="psum", bufs=2, space="PSUM") as psum, \
             tc.tile_pool(name="psum2", bufs=2, space="PSUM") as psum2:

            # ---- resident state ----
            def load(name, shape, dt, src):
                t = state.tile(shape, dt, tag=name)
                nc.sync.dma_start(out=t[:], in_=src[:].rearrange(
                    "p (a b) -> p a b", a=shape[1]) if len(shape) == 3 else src[:])
                return t

            xT = state.tile([P, KF, SPC], f16, tag="xT")
            nc.sync.dma_start(out=xT[:], in_=xT_in[:].rearrange("p (k s) -> p k s", k=KF))
            s16_t = state.tile([P, n_blk * 2 * CW], mybir.dt.int16, tag="s16")
            nc.sync.dma_start(out=s16_t[:], in_=s16_in[:])
            dst_t = state.tile([P, n_blk * T], f16, tag="dst")
            nc.sync.dma_start(out=dst_t[:], in_=dst_in[:])
            n12_t = state.tile([P, n_blk * T], f16, tag="n12")
            nc.sync.dma_start(out=n12_t[:], in_=n12_in[:])
            n3_t = state.tile([P, n_blk * T], f16, tag="n3")
            nc.sync.dma_start(out=n3_t[:], in_=n3_in[:])
            w1_t = state.tile([P, KF, NH], f16, tag="w1")
            nc.sync.dma_start(out=w1_t[:], in_=w1_in[:].rearrange("p (k n) -> p k n", k=KF))
            w2_t = state.tile([P, KH, NH], f16, tag="w2")
            nc.sync.dma_start(out=w2_t[:], in_=w2_in[:].rearrange("p (k n) -> p k n", k=KH))
            w3_t = state.tile([P, KH, NH], f16, tag="w3")
            nc.sync.dma_start(out=w3_t[:], in_=w3_in[:].rearrange("p (k n) -> p k n", k=KH))
            wl_t = state.tile([P, 3 * KH, NC], f16, tag="wl")
            nc.sync.dma_start(out=wl_t[:], in_=wl_in[:].rearrange("p (k n) -> p k n", k=3 * KH))
            b1_t = state.tile([P, KH], f32, tag="b1")
            nc.sync.dma_start(out=b1_t[:], in_=b1_in[:])
            b2_t = state.tile([P, KH], f32, tag="b2")
            nc.sync.dma_start(out=b2_t[:], in_=b2_in[:])
            b3_t = state.tile([P, KH], f32, tag="b3")
            nc.sync.dma_start(out=b3_t[:], in_=b3_in[:])
            bl_t = state.tile([P, NC], f32, tag="bl")
            nc.sync.dma_start(out=bl_t[:], in_=bl_in[:])
            iota = state.tile([P, P], f16, tag="iota")
            nc.gpsimd.iota(iota[:, :], pattern=[[1, P]], base=0,
                           channel_multiplier=0, allow_small_or_imprecise_dtypes=True)

            # activations (feature-major), persistent
            actT = [state.tile([P, KH, SPC], f16, tag=f"x{l}T", name=f"x{l}T") for l in (1, 2, 3)]

            def mm_phase(src_act, kk, w_t):
                """h = act @ W  -> hc DRAM (fp16). src_act [P, kk, SPC]."""
                for nb in range(n_blk):
                    ps = psum.tile([P, NH], f32, tag="mmps")
                    for k in range(kk):
                        nc.tensor.matmul(
                            ps[:],
                            lhsT=src_act[:, k, nb * P:(nb + 1) * P],
                            rhs=w_t[:, k, :],
                            start=(k == 0), stop=(k == kk - 1),
                        )
                    stage = sbuf.tile([P, NH], f16, tag="hstage")
                    nc.scalar.activation(stage[:], ps[:], AF.Copy)
                    nc.sync.dma_start(out=hc[nb * P:(nb + 1) * P, :], in_=stage[:])

            def edge_phase(nrm_t, bias_t, relu, outT):
                for b in range(n_blk):
                    g = sbuf.tile([P, T, NH], f16, tag="g")
                    for half in range(2):
                        nc.gpsimd.dma_gather(
                            out_ap=g[:, half * T_h:(half + 1) * T_h, :],
                            in_ap=hfull[half * HALF:(half + 1) * HALF, :],
                            idxs_ap=s16_t[:, (b * 2 + half) * CW:(b * 2 + half + 1) * CW],
                            num_idxs=NIH, num_idxs_reg=NIH, elem_size=NH,
                            single_packet=False, queue_num=(b * 2 + half) % 4,
                        )
                    nc.sync.dma_start(out=g[:, T_g, :], in_=hc[b * P:(b + 1) * P, :])
                    cols = slice(b * T, (b + 1) * T)
                    eq = spool.tile([P, T, P], f16, tag="eq")
                    nc.vector.tensor_tensor(
                        out=eq[:],
                        in0=dst_t[:, cols, None].to_broadcast([P, T, P]),
                        in1=iota[:, None, :].to_broadcast([P, T, P]),
                        op=ALU.is_equal,
                    )
                    S = spool.tile([P, T, P], f16, tag="S")
                    nc.vector.tensor_tensor(
                        out=S[:], in0=eq[:],
                        in1=nrm_t[:, cols, None].to_broadcast([P, T, P]),
                        op=ALU.mult,
                    )
                    pss = [psum2.tile([P, P], f32, tag=f"eps{h}", name=f"eps{h}")
                           for h in range(KH)]
                    for h in range(KH):
                        for t in range(T):
                            nc.tensor.matmul(
                                pss[h][:],
                                lhsT=g[:, t, h * P:(h + 1) * P],
                                rhs=S[:, t, :],
                                start=(t == 0), stop=(t == T - 1),
                            )
                    for h in range(KH):
                        nc.scalar.activation(
                            outT[:, h, b * P:(b + 1) * P], pss[h][:],
                            AF.Relu if relu else AF.Identity,
                            bias=bias_t[:, h:h + 1],
                        )

            def allgather():
                nc.gpsimd.collective_compute(
                    "AllGather", ALU.bypass, replica_groups=rg,
                    ins=[hc[:].opt()], outs=[hfull[:].opt()],
                )

            # ---- layer 1 ----
            mm_phase(xT, KF, w1_t)
            allgather()
            edge_phase(n12_t, b1_t, True, actT[0])
            # ---- layer 2 ----
            mm_phase(actT[0], KH, w2_t)
            allgather()
            edge_phase(n12_t, b2_t, True, actT[1])
            # ---- layer 3 ----
            mm_phase(actT[1], KH, w3_t)
            allgather()
            edge_phase(n3_t, b3_t, False, actT[2])

            # ---- final linear + log_softmax ----
            for nb in range(n_blk):
                ps = psum.tile([P, NC], f32, tag="fps")
                i = 0
                for xa in actT:
                    for k in range(KH):
                        nc.tensor.matmul(
                            ps[:], lhsT=xa[:, k, nb * P:(nb + 1) * P],
                            rhs=wl_t[:, i, :],
                            start=(i == 0), stop=(i == 3 * KH - 1),
                        )
                        i += 1
                lg = sbuf.tile([P, NC], f32, tag="lg")
                nc.vector.tensor_tensor(out=lg[:], in0=ps[:], in1=bl_t[:], op=ALU.add)
                mx = sbuf.tile([P, 1], f32, tag="mx")
                nc.vector.reduce_max(mx[:], lg[:], axis=mybir.AxisListType.X)
                sh = sbuf.tile([P, NC], f32, tag="sh")
                nc.vector.tensor_scalar(out=sh[:], in0=lg[:], scalar1=mx[:, :1],
                                        scalar2=None, op0=ALU.subtract)
                ex = sbuf.tile([P, NC], f32, tag="ex")
                nc.scalar.activation(ex[:], sh[:], AF.Exp)
                sm = sbuf.tile([P, 1], f32, tag="sm")
                nc.vector.reduce_sum(sm[:], ex[:], axis=mybir.AxisListType.X)
                lns = sbuf.tile([P, 1], f32, tag="lns")
                nc.scalar.activation(lns[:], sm[:], AF.Ln)
                res = sbuf.tile([P, NC], f32, tag="res")
                nc.vector.tensor_scalar(out=res[:], in0=sh[:], scalar1=lns[:, :1],
                                        scalar2=None, op0=ALU.subtract)
                nc.sync.dma_start(out=out_t[nb * P:(nb + 1) * P, :], in_=res[:])

    nc.compile()
    return nc


# ----------------------------------------------------------------------------
# Full pipeline: host entry
# ----------------------------------------------------------------------------

def make_in_maps(inputs, n_blk=None):
    pre, meta = preprocess(inputs["x"], np.asarray(inputs["edge_index"]),
                           np.asarray(inputs["edge_w_params"]), n_blk=n_blk)
    wts = prep_weights(inputs["W1"], inputs["b1"], inputs["W2"], inputs["b2"],
                       inputs["W3"], inputs["b3"], inputs["Wlin"], inputs["blin"])
    in_maps = []
    for c in range(N_CORES):
        m = dict(
            xT=pre["xT"][c].reshape(P, -1),
            src16=pre["src16"][c],
            dst_loc=pre["dst_loc"][c],
            nrm12=pre["nrm12"][c],
            nrm3=pre["nrm3"][c],
            w1=wts["w1"].reshape(P, -1), w2=wts["w2"].reshape(P, -1),
            w3=wts["w3"].reshape(P, -1), wlin=wts["wlin"].reshape(P, -1),
            b1h=wts["b1h"], b2h=wts["b2h"], b3h=wts["b3h"],
            blin_rep=wts["blin_rep"],
        )
        in_maps.append(m)
    return in_maps, meta


def unpermute(results, meta, n):
    slot = meta["slot"]
    spc = meta["slots_per_core"]
    full = np.concatenate([r["out"] for r in results], axis=0)  # [8*SPC, NC]
    return full[slot]


# ----------------------------------------------------------------------------
# Self-contained entry point (harness contract)
# ----------------------------------------------------------------------------

_CACHE = {}


def kernel(**inputs):
    """Full-input entry: shards across 8 NeuronCores internally, returns the
    full [n_nodes, n_class] log-softmax output (float32)."""
    inputs = {k: np.asarray(v) for k, v in inputs.items()}
    in_maps, meta = make_in_maps(inputs)
    key = (meta["n_blk"], meta["T_g"], inputs["x"].shape[1],
           inputs["W1"].shape[1], inputs["Wlin"].shape[1])
    if key not in _CACHE:
        _CACHE[key] = build_kernel(meta["n_blk"], meta["T_g"],
                                   NF=inputs["x"].shape[1],
                                   NH=inputs["W1"].shape[1],
                                   NC=inputs["Wlin"].shape[1])
    nc = _CACHE[key]
    from concourse.bass_utils import run_bass_kernel_spmd
    res = run_bass_kernel_spmd(nc, in_maps, core_ids=list(range(N_CORES)))
    out = unpermute(res.results, meta, inputs["x"].shape[0])
    return out.astype(np.float32)



# revision 7
# speedup vs baseline: 2.0997x; 2.0997x over previous
"""GCN (3-layer GCNConv + linear head + log_softmax) on 8 TRN2 NeuronCores.

v2 design:
- Nodes sharded across 8 cores (49 blocks of 128 slots per core, LPT-packed
  by in-degree).
- Per layer: local matmul h = act @ W (fp16 -> fp8 epilogue), h kept in SBUF
  (hlocal) for self-loop tiles and DMA'd to DRAM in two node-range chunks
  (hcA: blocks 0-24, hcB: blocks 25-48).
- Two chunked AllGathers per layer (fp8) overlap with the tail of the matmul
  and with the first chunk's edge processing.
- Edge aggregation per dst block: bundled indirect-DMA gathers (7 blocks per
  dma_gather call, fp8 rows), one-hot S matrix built on DVE with a single
  fused tensor_scalar per tile ((iota == dst) * norm, fp16), segment-sum via
  PE matmul (lhsT = gathered fp8 rows, rhs = fp16 S), ACT epilogue
  (bias + relu) into feature-major fp16 activations.
- Final linear + log_softmax: per-block PE matmuls into a batched [128,
  n_blk, 8] staging buffer; softmax chain runs batched (2 table loads total).
"""
import math
import numpy as np

import concourse.bass as bass
import concourse.bacc as bacc
import concourse.tile as tile
from concourse import mybir

P = 128
f16 = mybir.dt.float16
f32 = mybir.dt.float32
f8 = mybir.dt.float8e4
i16 = mybir.dt.int16
AF = mybir.ActivationFunctionType
ALU = mybir.AluOpType

N_CORES = 8
N_BLK = 49           # blocks of 128 node slots per core
G_BLK = 7            # dst blocks per dma_gather call
N_BUN = N_BLK // G_BLK
T_H = 5              # gather tiles per (block, src-chunk)
T_G = 2 * T_H
T = T_G + 1          # + self tile
NIH = T_H * P        # idxs per (block, chunk)
CWB = G_BLK * NIH // 16   # idx columns per bundled gather call
NA_BLK = 25          # chunk A = blocks [0, 25)
NB_BLK = N_BLK - NA_BLK
SA = NA_BLK * P      # 3200 chunk-A slots per core
SB = NB_BLK * P      # 3072


# ----------------------------------------------------------------------------
# Host-side preprocessing
# ----------------------------------------------------------------------------

def sigmoid64(x):
    return 1.0 / (1.0 + np.exp(-x.astype(np.float64)))


def pack_nodes(n_nodes, weights, n_bins, cap):
    """LPT-pack nodes into n_bins bins (<=cap nodes each), balancing sum of
    weights. Returns bin assignment [n_nodes]."""
    import heapq
    order = np.argsort(-weights, kind="stable")
    loads = np.zeros(n_bins)
    counts = np.zeros(n_bins, dtype=np.int64)
    assign = np.full(n_nodes, -1, dtype=np.int64)
    heap = [(0.0, b) for b in range(n_bins)]
    heapq.heapify(heap)
    full = set()
    for i in order:
        while True:
            load, b = heapq.heappop(heap)
            if b not in full:
                break
        assign[i] = b
        loads[b] = load + weights[i]
        counts[b] += 1
        if counts[b] >= cap:
            full.add(b)
        else:
            heapq.heappush(heap, (loads[b], b))
    return assign, loads


def preprocess(x, edge_index, edge_w_params, n_blk=None):
    n = x.shape[0]
    e_src = edge_index[0].astype(np.int64)
    e_dst = edge_index[1].astype(np.int64)

    ew = sigmoid64(edge_w_params)
    deg = np.zeros(n, dtype=np.float64)
    np.add.at(deg, e_dst, ew)
    deg += 1.0
    dinv = 1.0 / np.sqrt(deg)
    indeg = np.bincount(e_dst, minlength=n).astype(np.float64) + 1.0
    dinv3 = 1.0 / np.sqrt(indeg)

    norm12 = (dinv[e_src] * ew * dinv[e_dst]).astype(np.float32)
    norm3 = (dinv3[e_src] * dinv3[e_dst]).astype(np.float32)
    self12 = (dinv * dinv).astype(np.float32)
    self3 = (dinv3 * dinv3).astype(np.float32)

    w = np.bincount(e_dst, minlength=n).astype(np.float64)
    n_blk = N_BLK
    spc = n_blk * P

    # two-level packing: nodes -> 8 shards, then -> 49 blocks of <=128
    shard, _ = pack_nodes(n, w, N_CORES, spc)
    block = np.full(n, -1, dtype=np.int64)
    pos = np.full(n, -1, dtype=np.int64)
    for c in range(N_CORES):
        nodes_c = np.where(shard == c)[0]
        blk_c, _ = pack_nodes(len(nodes_c), w[nodes_c], n_blk, P)
        block[nodes_c] = blk_c
        for b in range(n_blk):
            nb = nodes_c[blk_c == b]
            pos[nb] = np.arange(len(nb))

    slot = shard * spc + block * P + pos  # [n] global output slot

    # per-core transposed features
    NF = x.shape[1]
    KF = NF // P
    xT = np.zeros((N_CORES, P, KF, spc), dtype=np.float16)
    xt_full = np.ascontiguousarray(x.T).astype(np.float16)
    for c in range(N_CORES):
        nodes_c = np.where(shard == c)[0]
        sl = block[nodes_c] * P + pos[nodes_c]
        xc = np.zeros((NF, spc), dtype=np.float16)
        xc[:, sl] = xt_full[:, nodes_c]
        xT[c] = xc.reshape(KF, P, spc).transpose(1, 0, 2)

    # src chunk-local gather indices (two AG chunks: blocks <25 / >=25)
    in_a = block[e_src] < NA_BLK
    loc_a = shard[e_src] * SA + block[e_src] * P + pos[e_src]
    loc_b = shard[e_src] * SB + (block[e_src] - NA_BLK) * P + pos[e_src]

    e_shard = shard[e_dst]
    e_block = block[e_dst]
    e_dloc = pos[e_dst].astype(np.float32)

    src16 = np.zeros((N_CORES, P, N_BUN * 2 * CWB), dtype=np.int16)
    dst_loc = np.zeros((N_CORES, P, n_blk * T), dtype=np.float32)
    nrm12_a = np.zeros((N_CORES, P, n_blk * T), dtype=np.float32)
    nrm3_a = np.zeros((N_CORES, P, n_blk * T), dtype=np.float32)

    for c in range(N_CORES):
        for k in range(N_BUN):
            for half in range(2):
                flat = np.zeros(G_BLK * NIH, dtype=np.int16)
                for j in range(G_BLK):
                    b = k * G_BLK + j
                    mall = np.where((e_shard == c) & (e_block == b))[0]
                    sel = in_a[mall] if half == 0 else ~in_a[mall]
                    m = mall[sel]
                    loc = (loc_a if half == 0 else loc_b)[m]
                    kk = len(m)
                    assert kk <= NIH, (c, b, half, kk)
                    order = np.argsort(loc, kind="stable")
                    m, loc = m[order], loc[order]
                    flat[j * NIH:j * NIH + kk] = loc.astype(np.int16)
                    sl = np.arange(kk)
                    tt = half * T_H + sl // P
                    pp = sl % P
                    colsT = b * T + tt
                    dst_loc[c, pp, colsT] = e_dloc[m]
                    nrm12_a[c, pp, colsT] = norm12[m]
                    nrm3_a[c, pp, colsT] = norm3[m]
                cw0 = (k * 2 + half) * CWB
                src16[c, :, cw0:cw0 + CWB] = np.tile(
                    flat.reshape(CWB, 16).T, (8, 1))
        # self tiles
        for b in range(n_blk):
            nodes_cb = np.where((shard == c) & (block == b))[0]
            pcb = pos[nodes_cb]
            col_self = b * T + T_G
            dst_loc[c, pcb, col_self] = pcb.astype(np.float32)
            nrm12_a[c, pcb, col_self] = self12[nodes_cb]
            nrm3_a[c, pcb, col_self] = self3[nodes_cb]

    meta = dict(slot=slot, n_blk=n_blk, T_g=T_G, slots_per_core=spc)
    return dict(xT=xT, src16=src16, dst_loc=dst_loc, nrm12=nrm12_a,
                nrm3=nrm3_a), meta


def prep_weights(W1, b1, W2, b2, W3, b3, Wlin, blin):
    def wt(W):
        K, N = W.shape
        return W.astype(np.float16).reshape(K // P, P, N).transpose(1, 0, 2).copy()
    def bh(b):
        return b.astype(np.float32).reshape(-1, P).T.copy()  # [128, KH]
    return dict(w1=wt(W1), w2=wt(W2), w3=wt(W3), wlin=wt(Wlin),
                b1h=bh(b1), b2h=bh(b2), b3h=bh(b3),
                blin_rep=np.tile(blin.astype(np.float32)[None, :], (P, 1)))


# ----------------------------------------------------------------------------
# Kernel builder
# ----------------------------------------------------------------------------

def build_kernel(n_blk=N_BLK, T_g=T_G, NF=512, NH=256, NC=8, sim_ag=False):
    assert n_blk == N_BLK and T_g == T_G
    KF = NF // P
    KH = NH // P

    nc = bacc.Bacc("TRN2", target_bir_lowering=False, debug=False,
                   num_devices=N_CORES, num_swdge_queues=4)

    SPC = n_blk * P

    xT_in = nc.dram_tensor("xT", [P, KF * SPC], f16, kind="ExternalInput")
    s16_in = nc.dram_tensor("src16", [P, N_BUN * 2 * CWB], i16, kind="ExternalInput")
    dst_in = nc.dram_tensor("dst_loc", [P, n_blk * T], f32, kind="ExternalInput")
    n12_in = nc.dram_tensor("nrm12", [P, n_blk * T], f32, kind="ExternalInput")
    n3_in = nc.dram_tensor("nrm3", [P, n_blk * T], f32, kind="ExternalInput")
    w1_in = nc.dram_tensor("w1", [P, KF * NH], f16, kind="ExternalInput")
    w2_in = nc.dram_tensor("w2", [P, KH * NH], f16, kind="ExternalInput")
    w3_in = nc.dram_tensor("w3", [P, KH * NH], f16, kind="ExternalInput")
    wl_in = nc.dram_tensor("wlin", [P, 3 * KH * NC], f16, kind="ExternalInput")
    b1_in = nc.dram_tensor("b1h", [P, KH], f32, kind="ExternalInput")
    b2_in = nc.dram_tensor("b2h", [P, KH], f32, kind="ExternalInput")
    b3_in = nc.dram_tensor("b3h", [P, KH], f32, kind="ExternalInput")
    bl_in = nc.dram_tensor("blin_rep", [P, NC], f32, kind="ExternalInput")
    out_t = nc.dram_tensor("out", [P, n_blk * NC], f32, kind="ExternalOutput")

    hcA = nc.dram_tensor("hcA", [SA, NH], f8, kind="Internal")
    hcB = nc.dram_tensor("hcB", [SB, NH], f8, kind="Internal")
    hfA = nc.dram_tensor("hfA", [N_CORES * SA, NH], f8, kind="Internal",
                         addr_space="Shared")
    hfB = nc.dram_tensor("hfB", [N_CORES * SB, NH], f8, kind="Internal",
                         addr_space="Shared")
    rg = [list(range(N_CORES))]

    with tile.TileContext(nc) as tc:
        with tc.tile_pool(name="state", bufs=1) as state, \
             tc.tile_pool(name="xpool", bufs=2) as xpool, \
             tc.tile_pool(name="gpool", bufs=2) as gpool, \
             tc.tile_pool(name="spool", bufs=3) as spool, \
             tc.tile_pool(name="mpool", bufs=3) as mpool, \
             tc.tile_pool(name="psum", bufs=2, space="PSUM") as psum, \
             tc.tile_pool(name="psum2", bufs=4, space="PSUM") as psum2:

            # ---- resident state ----
            s16_t = state.tile([P, N_BUN * 2 * CWB], i16, tag="s16")
            nc.sync.dma_start(out=s16_t[:], in_=s16_in[:])
            dst_t = state.tile([P, n_blk * T], f32, tag="dst")
            nc.sync.dma_start(out=dst_t[:], in_=dst_in[:])
            n12_t = state.tile([P, n_blk * T], f32, tag="n12")
            nc.sync.dma_start(out=n12_t[:], in_=n12_in[:])
            n3_t = state.tile([P, n_blk * T], f32, tag="n3")
            nc.sync.dma_start(out=n3_t[:], in_=n3_in[:])
            w1_t = state.tile([P, KF, NH], f16, tag="w1")
            nc.sync.dma_start(out=w1_t[:], in_=w1_in[:].rearrange("p (k n) -> p k n", k=KF))
            w2_t = state.tile([P, KH, NH], f16, tag="w2")
            nc.sync.dma_start(out=w2_t[:], in_=w2_in[:].rearrange("p (k n) -> p k n", k=KH))
            w3_t = state.tile([P, KH, NH], f16, tag="w3")
            nc.sync.dma_start(out=w3_t[:], in_=w3_in[:].rearrange("p (k n) -> p k n", k=KH))
            wl_t = state.tile([P, 3 * KH, NC], f16, tag="wl")
            nc.sync.dma_start(out=wl_t[:], in_=wl_in[:].rearrange("p (k n) -> p k n", k=3 * KH))
            b1_t = state.tile([P, KH], f32, tag="b1")
            nc.sync.dma_start(out=b1_t[:], in_=b1_in[:])
            b2_t = state.tile([P, KH], f32, tag="b2")
            nc.sync.dma_start(out=b2_t[:], in_=b2_in[:])
            b3_t = state.tile([P, KH], f32, tag="b3")
            nc.sync.dma_start(out=b3_t[:], in_=b3_in[:])
            bl_t = state.tile([P, NC], f32, tag="bl")
            nc.sync.dma_start(out=bl_t[:], in_=bl_in[:])
            iota = state.tile([P, P], f16, tag="iota")
            nc.gpsimd.iota(iota[:, :], pattern=[[1, P]], base=0,
                           channel_multiplier=0, allow_small_or_imprecise_dtypes=True)

            # persistent per-layer state (fp16: self-loop tiles need the
            # extra mantissa — fp8 self terms dominate the error budget)
            hloc = state.tile([P, n_blk, NH], f16, tag="hloc")
            actT = [state.tile([P, KH, SPC], f16, tag=f"x{l}T", name=f"x{l}T")
                    for l in (1, 2, 3)]
            res = state.tile([P, n_blk, NC], f32, tag="res")

            xT_v = xT_in[:].rearrange("p (k s) -> p k s", k=KF)

            def mm_block(nb, lhs_getter, kk, w_t):
                ps = psum.tile([P, NH], f32, tag="mmps")
                for k in range(kk):
                    nc.tensor.matmul(
                        ps[:], lhsT=lhs_getter(k, nb), rhs=w_t[:, k, :],
                        start=(k == 0), stop=(k == kk - 1),
                    )
                nc.scalar.activation(hloc[:, nb, :], ps[:], AF.Copy)

            def flush_hc(b0, b1):
                """Cast-copy hloc blocks [b0, b1) into hcA/hcB (fp16->fp8)."""
                if b0 >= b1:
                    return
                if b0 < NA_BLK:
                    e = min(b1, NA_BLK)
                    nc.gpsimd.dma_start(
                        out=hcA[b0 * P:e * P, :].rearrange(
                            "(a p) n -> p a n", p=P),
                        in_=hloc[:, b0:e, :])
                    b0 = e
                if b0 < b1:
                    a0, a1 = b0 - NA_BLK, b1 - NA_BLK
                    nc.gpsimd.dma_start(
                        out=hcB[a0 * P:a1 * P, :].rearrange(
                            "(a p) n -> p a n", p=P),
                        in_=hloc[:, b0:b1, :])

            def allgather(src, dstt):
                if sim_ag:
                    spc_r = src.shape[0]
                    for c in range(N_CORES):
                        nc.sync.dma_start(
                            out=dstt[c * spc_r:(c + 1) * spc_r, :], in_=src[:])
                    return
                nc.gpsimd.collective_compute(
                    "AllGather", ALU.bypass, replica_groups=rg,
                    ins=[src[:].opt()], outs=[dstt[:].opt()],
                )

            def mm_phase(layer, kk, w_t):
                flushed = 0

                def after_block(nb):
                    nonlocal flushed
                    if nb + 1 in (NA_BLK, n_blk) or (nb + 1) % G_BLK == 0:
                        flush_hc(flushed, nb + 1)
                        flushed = nb + 1
                    if nb == NA_BLK - 1:
                        allgather(hcA, hfA)
                    if nb == n_blk - 1:
                        allgather(hcB, hfB)

                if layer == 0:
                    for kb in range(N_BUN):
                        xc = xpool.tile([P, KF, G_BLK * P], f16, tag="xc")
                        nc.sync.dma_start(
                            out=xc[:],
                            in_=xT_v[:, :, kb * G_BLK * P:(kb + 1) * G_BLK * P])
                        for j in range(G_BLK):
                            nb = kb * G_BLK + j
                            mm_block(nb, lambda k, _nb, _xc=xc, _j=j:
                                     _xc[:, k, _j * P:(_j + 1) * P], kk, w_t)
                            after_block(nb)
                else:
                    src_act = actT[layer - 1]
                    for nb in range(n_blk):
                        mm_block(nb, lambda k, _nb:
                                 src_act[:, k, _nb * P:(_nb + 1) * P], kk, w_t)
                        after_block(nb)

            def edge_phase(nrm_t, bias_t, relu, outT):
                for kb in range(N_BUN):
                    gA = gpool.tile([P, G_BLK * T_H, NH], f8, tag="gA", name="gA")
                    nc.gpsimd.dma_gather(
                        out_ap=gA[:], in_ap=hfA[:],
                        idxs_ap=s16_t[:, (kb * 2) * CWB:(kb * 2 + 1) * CWB],
                        num_idxs=G_BLK * NIH, num_idxs_reg=G_BLK * NIH,
                        elem_size=NH, single_packet=False,
                        queue_num=(kb * 2) % 4,
                    )
                    gB = gpool.tile([P, G_BLK * T_H, NH], f8, tag="gB", name="gB")
                    nc.gpsimd.dma_gather(
                        out_ap=gB[:], in_ap=hfB[:],
                        idxs_ap=s16_t[:, (kb * 2 + 1) * CWB:(kb * 2 + 2) * CWB],
                        num_idxs=G_BLK * NIH, num_idxs_reg=G_BLK * NIH,
                        elem_size=NH, single_packet=False,
                        queue_num=(kb * 2 + 1) % 4,
                    )
                    for j in range(G_BLK):
                        b = kb * G_BLK + j
                        S = spool.tile([P, T, P], f16, tag="S")
                        for t in range(T):
                            col = b * T + t
                            nc.vector.tensor_scalar(
                                out=S[:, t, :], in0=iota[:, :],
                                scalar1=dst_t[:, col:col + 1],
                                scalar2=nrm_t[:, col:col + 1],
                                op0=ALU.is_equal, op1=ALU.mult,
                            )
                        for h in range(KH):
                            pss = psum2.tile([P, P], f32, tag="eps")
                            hs = slice(h * P, (h + 1) * P)
                            for t in range(T):
                                if t < T_H:
                                    lhsT = gA[:, j * T_H + t, hs]
                                elif t < T_G:
                                    lhsT = gB[:, j * T_H + (t - T_H), hs]
                                else:
                                    lhsT = hloc[:, b, hs]
                                nc.tensor.matmul(
                                    pss[:], lhsT=lhsT, rhs=S[:, t, :],
                                    start=(t == 0), stop=(t == T - 1),
                                )
                            nc.scalar.activation(
                                outT[:, h, b * P:(b + 1) * P], pss[:],
                                AF.Relu if relu else AF.Identity,
                                bias=bias_t[:, h:h + 1],
                            )

            # ---- layers ----
            mm_phase(0, KF, w1_t)
            edge_phase(n12_t, b1_t, True, actT[0])
            mm_phase(1, KH, w2_t)
            edge_phase(n12_t, b2_t, True, actT[1])
            mm_phase(2, KH, w3_t)
            edge_phase(n3_t, b3_t, False, actT[2])

            # ---- final linear (per block) + batched log_softmax ----
            for nb in range(n_blk):
                ps = psum.tile([P, NC], f32, tag="fps")
                i = 0
                for xa in actT:
                    for k in range(KH):
                        nc.tensor.matmul(
                            ps[:], lhsT=xa[:, k, nb * P:(nb + 1) * P],
                            rhs=wl_t[:, i, :],
                            start=(i == 0), stop=(i == 3 * KH - 1),
                        )
                        i += 1
                nc.vector.tensor_tensor(out=res[:, nb, :], in0=ps[:],
                                        in1=bl_t[:], op=ALU.add)

            mx = mpool.tile([P, n_blk], f32, tag="mx")
            nc.vector.reduce_max(mx[:], res[:], axis=mybir.AxisListType.X)
            sh = mpool.tile([P, n_blk, NC], f32, tag="sh")
            nc.vector.tensor_tensor(
                out=sh[:], in0=res[:],
                in1=mx[:, :, None].to_broadcast([P, n_blk, NC]),
                op=ALU.subtract)
            ex = mpool.tile([P, n_blk, NC], f32, tag="ex")
            nc.scalar.activation(ex[:], sh[:], AF.Exp)
            sm = mpool.tile([P, n_blk], f32, tag="sm")
            nc.vector.reduce_sum(sm[:], ex[:], axis=mybir.AxisListType.X)
            lns = mpool.tile([P, n_blk], f32, tag="lns")
            nc.scalar.activation(lns[:], sm[:], AF.Ln)
            fin = mpool.tile([P, n_blk, NC], f32, tag="fin")
            nc.vector.tensor_tensor(
                out=fin[:], in0=sh[:],
                in1=lns[:, :, None].to_broadcast([P, n_blk, NC]),
                op=ALU.subtract)
            nc.sync.dma_start(out=out_t[:],
                              in_=fin[:].rearrange("p a b -> p (a b)"))

    nc.compile()
    return nc


# ----------------------------------------------------------------------------
# Full pipeline: host entry
# ----------------------------------------------------------------------------

def make_in_maps(inputs, n_blk=None):
    pre, meta = preprocess(inputs["x"], np.asarray(inputs["edge_index"]),
                           np.asarray(inputs["edge_w_params"]))
    wts = prep_weights(inputs["W1"], inputs["b1"], inputs["W2"], inputs["b2"],
                       inputs["W3"], inputs["b3"], inputs["Wlin"], inputs["blin"])
    in_maps = []
    for c in range(N_CORES):
        m = dict(
            xT=pre["xT"][c].reshape(P, -1),
            src16=pre["src16"][c],
            dst_loc=pre["dst_loc"][c],
            nrm12=pre["nrm12"][c],
            nrm3=pre["nrm3"][c],
            w1=wts["w1"].reshape(P, -1), w2=wts["w2"].reshape(P, -1),
            w3=wts["w3"].reshape(P, -1), wlin=wts["wlin"].reshape(P, -1),
            b1h=wts["b1h"], b2h=wts["b2h"], b3h=wts["b3h"],
            blin_rep=wts["blin_rep"],
        )
        in_maps.append(m)
    return in_maps, meta


def unpermute(results, meta, n):
    slot = meta["slot"]
    spc = meta["slots_per_core"]
    n_blk = meta["n_blk"]
    cores = []
    for r in results:
        a = r["out"].reshape(P, n_blk, -1)          # [pos, block, class]
        cores.append(np.transpose(a, (1, 0, 2)).reshape(spc, -1))
    full = np.concatenate(cores, axis=0)            # [8*SPC, NC]
    return full[slot]


# ----------------------------------------------------------------------------
# Self-contained entry point (harness contract)
# ----------------------------------------------------------------------------

_CACHE = {}


def kernel(**inputs):
    """Full-input entry: shards across 8 NeuronCores internally, returns the
    full [n_nodes, n_class] log-softmax output (float32)."""
    inputs = {k: np.asarray(v) for k, v in inputs.items()}
    in_maps, meta = make_in_maps(inputs)
    key = (meta["n_blk"], meta["T_g"], inputs["x"].shape[1],
           inputs["W1"].shape[1], inputs["Wlin"].shape[1])
    if key not in _CACHE:
        _CACHE[key] = build_kernel(meta["n_blk"], meta["T_g"],
                                   NF=inputs["x"].shape[1],
                                   NH=inputs["W1"].shape[1],
                                   NC=inputs["Wlin"].shape[1])
    nc = _CACHE[key]
    from concourse.bass_utils import run_bass_kernel_spmd
    res = run_bass_kernel_spmd(nc, in_maps, core_ids=list(range(N_CORES)))
    out = unpermute(res.results, meta, inputs["x"].shape[0])
    return out.astype(np.float32)
